# revision 6
# baseline (speedup 1.0000x reference)
"""Trainium2 Bass kernel: document-level LSTM (B=64, T=1024, D=300, H=512)
with mean-over-time pooling and a sigmoid dense head.

Strategy (8 NeuronCores, TIME-sharded):

  The LSTM forget gate makes the recurrence exponentially forgetting
  (per-step cell decay sigma(f+1), E[ln sigma] ~ -0.4), so the scan can be
  split over time: core c runs steps [128c - W, 128c + 128) with h=c=0 at
  the window start and discards the first W warm-up outputs.  Truncation
  error decays like e^{-0.4 W}; W=64 puts it far below fp16 noise.  Core 0's
  window is padded with W all-zero inputs (zero state is a fixed point of
  the gate math since the j-gate bias is zero), so a single SPMD program
  runs on all cores.  Each core therefore executes 192 sequential steps
  instead of 1024 with the FULL batch of 64, which amortizes the per-step
  Wh weight-load cost (the bottleneck) 8x better than batch-sharding.

  Everything on-chip is gate-major: gate tensors live as [128 partitions =
  position-within-128-chunk, free = (chunk, batch)], and the state h is kept
  as h.T tiles [128, (k-chunk, batch)] -- exactly the moving operand the
  recurrence matmul needs, so there are no transposes.

  Per step, gates.T[m] = sum_k Wh[k,m].T @ h.T[k]: fixed Wh tiles [128,128]
  (fp16, FastWeightLoad) stationary, h.T [128,64] slices moving.  The input
  projection x@Wx (+bias via a constant-1 row folded into ex/Wx) is computed
  in 16-step blocks fused into the loop in the same gate-major layout and
  seeded into each gate's PSUM bank with an identity matmul before the
  h-dependent matmuls, so the activations read PSUM directly.  Gate order
  along the 16 m-chunks is [f, i, j, o] so the c-update chain starts early.

  Mean-over-time is a running fp32 accumulator over the 128 payload steps;
  each core emits its partial dense dot acc@W_dense [64,1] and the host
  sums across cores and applies the final sigmoid.
"""
import sys
sys.path.insert(0, "/opt/trn_rl_repo")
import numpy as np

B = 64         # full batch on every core (time-sharded)
BF = 64        # full batch
H = 512
G4 = 2048
D = 300
D_PAD = 384
T = 1024
W_UP = 32      # warm-up steps (outputs discarded)
T_CHUNK = 128  # payload steps per core
T_LOC = W_UP + T_CHUNK
BS = 16        # phase-1 time block size
NB = T_LOC // BS
KC = 4         # H / 128
MC = 16        # 4H / 128
N_CORES = 8

_CACHE = {}


def _build():
    import concourse.mybir as mybir
    import concourse.tile as tile
    from concourse import bacc

    F32 = mybir.dt.float32
    F16 = mybir.dt.float16
    AF = mybir.ActivationFunctionType
    OP = mybir.AluOpType

    nc = bacc.Bacc("TRN2", target_bir_lowering=False, debug=False,
                   num_devices=N_CORES)

    # ex is pre-transposed host-side: [k-chunk, d-in-chunk, (t, b)]
    ex_d = nc.dram_tensor("ex", [3, 128, T_LOC * B], F16, kind="ExternalInput")
    ident_d = nc.dram_tensor("ident", [128, 128], F16, kind="ExternalInput")
    wh_d = nc.dram_tensor("wh", [128, KC * MC * 128], F16, kind="ExternalInput")
    wx_d = nc.dram_tensor("wx", [128, 3 * MC * 128], F16, kind="ExternalInput")
    wd_d = nc.dram_tensor("wd", [128, KC], F32, kind="ExternalInput")
    out_d = nc.dram_tensor("out", [B, 1], F32, kind="ExternalOutput")

    with tile.TileContext(nc) as tc:
        with (
            tc.tile_pool(name="w", bufs=1) as wpool,
            tc.tile_pool(name="xp", bufs=2) as xppool,
            tc.tile_pool(name="ex", bufs=2) as expool,
            tc.tile_pool(name="ew", bufs=4) as ewpool,
            tc.tile_pool(name="st", bufs=4) as stpool,
            tc.tile_pool(name="p1", bufs=2, space="PSUM") as p1pool,
            tc.tile_pool(name="pg", bufs=1, space="PSUM") as gpool,
            tc.tile_pool(name="pd", bufs=1, space="PSUM") as pdpool,
        ):
            wh = wpool.tile([128, KC * MC * 128], F16)
            wx = wpool.tile([128, 3 * MC * 128], F16)
            wd = wpool.tile([128, KC], F32)
            nc.sync.dma_start(out=wh[:], in_=wh_d[:])
            nc.sync.dma_start(out=wx[:], in_=wx_d[:])
            nc.sync.dma_start(out=wd[:], in_=wd_d[:])
            ident = wpool.tile([128, 128], F16, tag="ident", name="ident")
            nc.sync.dma_start(out=ident[:], in_=ident_d[:])

            h = stpool.tile([128, KC * B], F16, tag="h")
            c = stpool.tile([128, KC * B], F32, tag="c")
            acc = stpool.tile([128, KC * B], F32, tag="acc")
            nc.vector.memset(h[:], 0.0)
            nc.vector.memset(c[:], 0.0)
            nc.vector.memset(acc[:], 0.0)

            def load_ex(bb):
                t0 = bb * BS
                tiles = []
                for k in range(3):
                    et = expool.tile([128, BS * B], F16, tag=f"ex{k}",
                                     name=f"ex{k}")
                    nc.sync.dma_start(out=et[:],
                                      in_=ex_d[k, :, t0 * B:(t0 + BS) * B])
                    tiles.append(et)
                return tiles

            def phase1_mgroup(xp_t, ex_tiles, m):
                # one m-chunk of x@Wx for a BS-step block: 2 PSUM halves of
                # 512 cols (8 steps x 64 batch each), one weight load per k
                ps_a = p1pool.tile([128, 512], F32, tag="p1", name="p1a")
                ps_b = p1pool.tile([128, 512], F32, tag="p1", name="p1b")
                for k in range(3):
                    w_sl = wx[:, (k * MC + m) * 128:(k * MC + m + 1) * 128]
                    nc.tensor.matmul(ps_a[:], w_sl, ex_tiles[k][:, :512],
                                     start=(k == 0), stop=(k == 2),
                                     skip_group_check=True)
                    nc.tensor.matmul(ps_b[:], w_sl, ex_tiles[k][:, 512:],
                                     start=(k == 0), stop=(k == 2),
                                     skip_group_check=True)
                xv = xp_t[:].rearrange("p (t m b) -> p t m b", t=BS, m=MC, b=B)
                av = ps_a[:].rearrange("p (t b) -> p t b", t=BS // 2, b=B)
                bv = ps_b[:].rearrange("p (t b) -> p t b", t=BS // 2, b=B)
                nc.vector.tensor_copy(out=xv[:, :BS // 2, m, :], in_=av[:])
                nc.vector.tensor_copy(out=xv[:, BS // 2:, m, :], in_=bv[:])

            ex_tiles = load_ex(0)
            xp_cur = xppool.tile([128, BS * MC * B], F16, tag="xp", name="xp")
            for m in range(MC):
                phase1_mgroup(xp_cur, ex_tiles, m)
            xp_next = None

            for t in range(T_LOC):
                bb, tloc = divmod(t, BS)
                if tloc == 0 and bb + 1 < NB:
                    ex_tiles = load_ex(bb + 1)
                    xp_next = xppool.tile([128, BS * MC * B], F16, tag="xp",
                                          name="xp")
                if bb + 1 < NB:
                    phase1_mgroup(xp_next, ex_tiles, tloc)

                sig = {}
                cf = u = c_new = tanh_c = None
                # seed all four gates' PSUM with xp (ps = I.T @ xp_slice)
                # BEFORE any h-dependent matmul: the PE queue is in-order, so
                # the seeds (and phase-1 work above) execute during the
                # previous step's elementwise tail.
                ps_g = []
                for g in range(4):
                    ps = gpool.tile([128, 4 * B], F32, tag=f"pg{g}",
                                    name=f"pg{g}", padded_shape=[128, 512])
                    xp_slice = xp_cur[:, (tloc * MC + g * 4) * B:
                                      (tloc * MC + (g + 1) * 4) * B]
                    nc.tensor.matmul(ps[:], ident[:], xp_slice,
                                     start=True, stop=False,
                                     skip_group_check=True)
                    ps_g.append(ps)
                for g in range(4):  # gate order: f, i, j, o
                    ps = ps_g[g]
                    for mm in range(4):
                        m = g * 4 + mm
                        for k in range(KC):
                            nc.tensor.matmul(
                                ps[:, mm * B:(mm + 1) * B],
                                wh[:, (k * MC + m) * 128:(k * MC + m + 1) * 128],
                                h[:, k * B:(k + 1) * B],
                                start=False, stop=(k == KC - 1),
                                skip_group_check=True,
                            )
                    st = ewpool.tile([128, 4 * B], F32, tag=f"s{g}",
                                     name=f"s{g}")
                    nc.scalar.activation(out=st[:], in_=ps[:],
                                         func=AF.Tanh if g == 2 else AF.Sigmoid)
                    sig[g] = st
                    if g == 0:
                        # on GPSIMD (otherwise idle; SBUF-only operands) so it
                        # runs concurrently with the DVE's u = sig(i)*tanh(j)
                        cf = ewpool.tile([128, 4 * B], F32, tag="cf", name="cf")
                        nc.gpsimd.tensor_tensor(cf[:], c[:], st[:], OP.mult)
                    elif g == 2:
                        u = ewpool.tile([128, 4 * B], F32, tag="u", name="u")
                        nc.vector.tensor_tensor(u[:], sig[1][:], st[:], OP.mult)
                        c_new = stpool.tile([128, KC * B], F32, tag="c",
                                            name="c")
                        nc.vector.tensor_tensor(c_new[:], cf[:], u[:], OP.add)
                        tanh_c = ewpool.tile([128, 4 * B], F32, tag="tc",
                                             name="tc")
                        nc.scalar.activation(out=tanh_c[:], in_=c_new[:],
                                             func=AF.Tanh)
                h_new = stpool.tile([128, KC * B], F16, tag="h", name="h")
                nc.vector.tensor_tensor(h_new[:], tanh_c[:], sig[3][:], OP.mult)
                if t >= W_UP:
                    acc_new = stpool.tile([128, KC * B], F32, tag="acc",
                                          name="acc")
                    nc.gpsimd.tensor_tensor(acc_new[:], acc[:], h_new[:],
                                            OP.add)
                    acc = acc_new
                h, c = h_new, c_new

                if tloc == BS - 1 and bb + 1 < NB:
                    xp_cur = xp_next

            pd = pdpool.tile([B, 1], F32, tag="pd")
            for k in range(KC):
                nc.tensor.matmul(pd[:], acc[:, k * B:(k + 1) * B],
                                 wd[:, k:k + 1],
                                 start=(k == 0), stop=(k == KC - 1))
            res = ewpool.tile([B, 1], F32, tag="res")
            nc.vector.tensor_copy(out=res[:], in_=pd[:])
            nc.sync.dma_start(out=out_d[:], in_=res[:])

    nc.compile()
    return nc


def _get_exec():
    if "exec" in _CACHE:
        return _CACHE["exec"]
    import jax
    import concourse.mybir as mybir
    from concourse import bass2jax
    from jax.sharding import Mesh, PartitionSpec, NamedSharding
    from jax.experimental.shard_map import shard_map

    nc = _build()
    bass2jax.install_neuronx_cc_hook()
    partition_name = (nc.partition_id_tensor.name
                      if nc.partition_id_tensor else None)
    in_names, out_names, out_avals = [], [], []
    for alloc in nc.m.functions[0].allocations:
        if not isinstance(alloc, mybir.MemoryLocationSet):
            continue
        name = alloc.memorylocations[0].name
        if alloc.kind == "ExternalInput":
            if name != partition_name:
                in_names.append(name)
        elif alloc.kind == "ExternalOutput":
            out_names.append(name)
            out_avals.append(jax.core.ShapedArray(
                tuple(alloc.tensor_shape), mybir.dt.np(alloc.dtype)))
    n_params = len(in_names)
    all_in = in_names + out_names + ([partition_name] if partition_name else [])

    def _body(*a):
        operands = list(a)
        if partition_name is not None:
            operands.append(bass2jax.partition_id_tensor())
        return tuple(bass2jax._bass_exec_p.bind(
            *operands, out_avals=tuple(out_avals), in_names=tuple(all_in),
            out_names=tuple(out_names), lowering_input_output_aliases=(),
            sim_require_finite=True, sim_require_nnan=True, nc=nc))

    devices = jax.devices()[:N_CORES]
    mesh = Mesh(np.asarray(devices), ("core",))
    jitted = jax.jit(
        shard_map(_body, mesh=mesh,
                  in_specs=(PartitionSpec("core"),) * (n_params + len(out_avals)),
                  out_specs=(PartitionSpec("core"),) * len(out_names),
                  check_rep=False),
        keep_unused=True)
    shard = NamedSharding(mesh, PartitionSpec("core"))
    state = (jitted, in_names, out_avals, mesh, shard)
    _CACHE["exec"] = state
    return state


def _prep_in_maps(essays, W_lstm, b_lstm, W_dense, b_dense):
    perm = np.concatenate([
        np.arange(1024, 1536),   # f
        np.arange(0, 512),       # i
        np.arange(512, 1024),    # j
        np.arange(1536, 2048),   # o
    ])
    Wx = W_lstm[:D][:, perm]
    Wh = W_lstm[D:][:, perm]
    b_eff = b_lstm[perm].astype(np.float32).copy()
    b_eff[0:512] += 1.0  # TF BasicLSTMCell forget bias ([f] block is first)

    Wx_pad = np.zeros((D_PAD, G4), np.float32)
    Wx_pad[:D] = Wx
    Wx_pad[D] = b_eff  # bias row, matched by constant-1 column in ex
    wx_packed = Wx_pad.reshape(3, 128, MC, 128).transpose(1, 0, 2, 3) \
        .reshape(128, 3 * MC * 128).astype(np.float16)
    wh_packed = Wh.reshape(KC, 128, MC, 128).transpose(1, 0, 2, 3) \
        .reshape(128, KC * MC * 128).astype(np.float16)
    wd_t = W_dense[:, 0].reshape(KC, 128).T.copy().astype(np.float32)

    # global time-padded input: W_UP zero steps (zero state is a fixed
    # point), then essays with the constant-1 bias column
    ex_glob = np.zeros((BF, W_UP + T, D_PAD), np.float16)
    ex_glob[:, W_UP:, :D] = essays.astype(np.float16)
    ex_glob[:, W_UP:, D] = 1.0

    ident = np.eye(128, dtype=np.float16)
    in_maps = []
    for core in range(N_CORES):
        # core's window in padded time coords: [128c, 128c + T_LOC)
        win = ex_glob[:, 128 * core:128 * core + T_LOC]      # [B, T_LOC, 384]
        # -> [k-chunk, d-in-chunk, t, b]
        ex_t = np.ascontiguousarray(
            win.transpose(2, 1, 0).reshape(3, 128, T_LOC * B))
        in_maps.append({
            "ex": ex_t,
            "wh": wh_packed,
            "wx": wx_packed,
            "wd": wd_t,
            "ident": ident,
        })
    return in_maps


def _finish(out, b_dense):
    # out[0]: [N_CORES*B, 1] partial dense dots; sum over cores, mean over
    # time, add bias, sigmoid
    pd = np.asarray(out[0]).reshape(N_CORES, BF).sum(axis=0)
    logits = pd / T + float(b_dense[0])
    return (1.0 / (1.0 + np.exp(-logits))).astype(np.float32)


def kernel(essays, W_lstm, b_lstm, W_dense, b_dense):
    import jax
    essays = np.asarray(essays, np.float32)
    W_lstm = np.asarray(W_lstm, np.float32)
    b_lstm = np.asarray(b_lstm, np.float32)
    W_dense = np.asarray(W_dense, np.float32)
    b_dense = np.asarray(b_dense, np.float32)

    jitted, in_names, out_avals, mesh, shard = _get_exec()
    in_maps = _prep_in_maps(essays, W_lstm, b_lstm, W_dense, b_dense)
    concat_in = [np.concatenate([in_maps[c][nm] for c in range(N_CORES)],
                                axis=0) for nm in in_names]
    concat_zeros = [np.zeros((N_CORES * a.shape[0], *a.shape[1:]), a.dtype)
                    for a in out_avals]
    dev_in = [jax.device_put(a, shard) for a in concat_in]
    dev_zeros = [jax.device_put(a, shard) for a in concat_zeros]
    out = jitted(*dev_in, *dev_zeros)
    jax.block_until_ready(out)
    return _finish(out, b_dense)


# expose the device-resident runner for timing harnesses
def _timed_run(essays, W_lstm, b_lstm, W_dense, b_dense, n_launch=9,
               trials=4):
    """Return (preds, per_launch_seconds_median) using pipelined launches."""
    import time, jax
    jitted, in_names, out_avals, mesh, shard = _get_exec()
    b_dense = np.asarray(b_dense, np.float32)
    in_maps = _prep_in_maps(np.asarray(essays, np.float32),
                            np.asarray(W_lstm, np.float32),
                            np.asarray(b_lstm, np.float32),
                            np.asarray(W_dense, np.float32),
                            b_dense)
    concat_in = [np.concatenate([in_maps[c][nm] for c in range(N_CORES)],
                                axis=0) for nm in in_names]
    concat_zeros = [np.zeros((N_CORES * a.shape[0], *a.shape[1:]), a.dtype)
                    for a in out_avals]
    dev_in = [jax.device_put(a, shard) for a in concat_in]
    dev_zeros = [jax.device_put(a, shard) for a in concat_zeros]

    out = jitted(*dev_in, *dev_zeros)
    jax.block_until_ready(out)
    preds = _finish(out, b_dense)

    def timed(K):
        t0 = time.perf_counter()
        o = None
        for _ in range(K):
            o = jitted(*dev_in, *dev_zeros)
        jax.block_until_ready(o)
        return time.perf_counter() - t0

    # pipelined-launch slope: marginal cost of 16 extra launches.  This is
    # an upper bound on device time (host dispatch overlaps device exec).
    timed(2)  # warm
    margins = []
    for _ in range(trials):
        t3 = timed(3)
        t19 = timed(19)
        margins.append((t19 - t3) / 16)
    return preds, float(np.median(margins))


# revision 10
# speedup vs baseline: 1.0316x; 1.0316x over previous
"""Trainium2 Bass kernel: document-level LSTM (B=64, T=1024, D=300, H=512)
with mean-over-time pooling and a sigmoid dense head.

Strategy (8 NeuronCores, TIME-sharded):

  The LSTM forget gate makes the recurrence exponentially forgetting
  (per-step cell decay sigma(f+1), E[ln sigma] ~ -0.4), so the scan can be
  split over time: core c runs steps [128c - W, 128c + 128) with h=c=0 at
  the window start and discards the first W warm-up outputs.  Truncation
  error decays like e^{-0.4 W}; W=64 puts it far below fp16 noise.  Core 0's
  window is padded with W all-zero inputs (zero state is a fixed point of
  the gate math since the j-gate bias is zero), so a single SPMD program
  runs on all cores.  Each core therefore executes 192 sequential steps
  instead of 1024 with the FULL batch of 64, which amortizes the per-step
  Wh weight-load cost (the bottleneck) 8x better than batch-sharding.

  Everything on-chip is gate-major: gate tensors live as [128 partitions =
  position-within-128-chunk, free = (chunk, batch)], and the state h is kept
  as h.T tiles [128, (k-chunk, batch)] -- exactly the moving operand the
  recurrence matmul needs, so there are no transposes.

  Per step, gates.T[m] = sum_k Wh[k,m].T @ h.T[k]: fixed Wh tiles [128,128]
  (fp16, FastWeightLoad) stationary, h.T [128,64] slices moving.  The input
  projection x@Wx (+bias via a constant-1 row folded into ex/Wx) is computed
  in 16-step blocks fused into the loop in the same gate-major layout and
  seeded into each gate's PSUM bank with an identity matmul before the
  h-dependent matmuls, so the activations read PSUM directly.  Gate order
  along the 16 m-chunks is [f, i, j, o] so the c-update chain starts early.

  Mean-over-time is a running fp32 accumulator over the 128 payload steps;
  each core emits its partial dense dot acc@W_dense [64,1] and the host
  sums across cores and applies the final sigmoid.
"""
import sys
sys.path.insert(0, "/opt/trn_rl_repo")
import numpy as np

B = 64         # full batch on every core (time-sharded)
BF = 64        # full batch
H = 512
G4 = 2048
D = 300
D_PAD = 384
T = 1024
W_UP = 32      # warm-up steps (outputs discarded)
T_CHUNK = 128  # payload steps per core
T_LOC = W_UP + T_CHUNK
BS = 16        # phase-1 time block size
NB = T_LOC // BS
KC = 4         # H / 128
MC = 16        # 4H / 128
N_CORES = 8

_CACHE = {}


def _build():
    import concourse.mybir as mybir
    import concourse.tile as tile
    from concourse import bacc

    F32 = mybir.dt.float32
    F16 = mybir.dt.float16
    F8 = mybir.dt.float8e4
    AF = mybir.ActivationFunctionType
    OP = mybir.AluOpType

    nc = bacc.Bacc("TRN2", target_bir_lowering=False, debug=False,
                   num_devices=N_CORES)

    # ex is pre-transposed host-side: [k-chunk, d-in-chunk, (t, b)]
    ex_d = nc.dram_tensor("ex", [3, 128, T_LOC * B], F16, kind="ExternalInput")
    ident_d = nc.dram_tensor("ident", [128, 128], F16, kind="ExternalInput")
    wh_d = nc.dram_tensor("wh", [128, KC * MC * 128], F8, kind="ExternalInput")
    wx_d = nc.dram_tensor("wx", [128, 3 * MC * 128], F16, kind="ExternalInput")
    wd_d = nc.dram_tensor("wd", [128, KC], F32, kind="ExternalInput")
    out_d = nc.dram_tensor("out", [B, 1], F32, kind="ExternalOutput")

    with tile.TileContext(nc) as tc:
        with (
            tc.tile_pool(name="w", bufs=1) as wpool,
            tc.tile_pool(name="xp", bufs=2) as xppool,
            tc.tile_pool(name="ex", bufs=2) as expool,
            tc.tile_pool(name="ew", bufs=4) as ewpool,
            tc.tile_pool(name="st", bufs=4) as stpool,
            tc.tile_pool(name="p1", bufs=2, space="PSUM") as p1pool,
            tc.tile_pool(name="pg", bufs=1, space="PSUM") as gpool,
            tc.tile_pool(name="pd", bufs=1, space="PSUM") as pdpool,
        ):
            wh = wpool.tile([128, KC * MC * 128], F8)
            wx = wpool.tile([128, 3 * MC * 128], F16)
            wd = wpool.tile([128, KC], F32)
            nc.sync.dma_start(out=wh[:], in_=wh_d[:])
            nc.sync.dma_start(out=wx[:], in_=wx_d[:])
            nc.sync.dma_start(out=wd[:], in_=wd_d[:])
            ident = wpool.tile([128, 128], F16, tag="ident", name="ident")
            nc.sync.dma_start(out=ident[:], in_=ident_d[:])

            h = stpool.tile([128, KC * B], F16, tag="h")
            c = stpool.tile([128, KC * B], F32, tag="c")
            acc = stpool.tile([128, KC * B], F32, tag="acc")
            nc.vector.memset(h[:], 0.0)
            nc.vector.memset(c[:], 0.0)
            nc.vector.memset(acc[:], 0.0)

            def load_ex(bb):
                t0 = bb * BS
                tiles = []
                for k in range(3):
                    et = expool.tile([128, BS * B], F16, tag=f"ex{k}",
                                     name=f"ex{k}")
                    nc.sync.dma_start(out=et[:],
                                      in_=ex_d[k, :, t0 * B:(t0 + BS) * B])
                    tiles.append(et)
                return tiles

            def phase1_mgroup(xp_t, ex_tiles, m):
                # one m-chunk of x@Wx for a BS-step block: 2 PSUM halves of
                # 512 cols (8 steps x 64 batch each), one weight load per k
                ps_a = p1pool.tile([128, 512], F32, tag="p1", name="p1a")
                ps_b = p1pool.tile([128, 512], F32, tag="p1", name="p1b")
                for k in range(3):
                    w_sl = wx[:, (k * MC + m) * 128:(k * MC + m + 1) * 128]
                    nc.tensor.matmul(ps_a[:], w_sl, ex_tiles[k][:, :512],
                                     start=(k == 0), stop=(k == 2),
                                     skip_group_check=True)
                    nc.tensor.matmul(ps_b[:], w_sl, ex_tiles[k][:, 512:],
                                     start=(k == 0), stop=(k == 2),
                                     skip_group_check=True)
                xv = xp_t[:].rearrange("p (t m b) -> p t m b", t=BS, m=MC, b=B)
                av = ps_a[:].rearrange("p (t b) -> p t b", t=BS // 2, b=B)
                bv = ps_b[:].rearrange("p (t b) -> p t b", t=BS // 2, b=B)
                nc.vector.tensor_copy(out=xv[:, :BS // 2, m, :], in_=av[:])
                nc.vector.tensor_copy(out=xv[:, BS // 2:, m, :], in_=bv[:])

            ex_tiles = load_ex(0)
            xp_cur = xppool.tile([128, BS * MC * B], F16, tag="xp", name="xp")
            for m in range(MC):
                phase1_mgroup(xp_cur, ex_tiles, m)
            xp_next = None

            for t in range(T_LOC):
                bb, tloc = divmod(t, BS)
                if tloc == 0 and bb + 1 < NB:
                    ex_tiles = load_ex(bb + 1)
                    xp_next = xppool.tile([128, BS * MC * B], F16, tag="xp",
                                          name="xp")
                if bb + 1 < NB:
                    phase1_mgroup(xp_next, ex_tiles, tloc)

                sig = {}
                cf = u = c_new = tanh_c = None
                # seed all four gates' PSUM with xp (ps = I.T @ xp_slice)
                # BEFORE any h-dependent matmul: the PE queue is in-order, so
                # the seeds (and phase-1 work above) execute during the
                # previous step's elementwise tail.
                ps_g = []
                for g in range(4):
                    ps = gpool.tile([128, 4 * B], F32, tag=f"pg{g}",
                                    name=f"pg{g}", padded_shape=[128, 512])
                    xp_slice = xp_cur[:, (tloc * MC + g * 4) * B:
                                      (tloc * MC + (g + 1) * 4) * B]
                    nc.tensor.matmul(ps[:], ident[:], xp_slice,
                                     start=True, stop=False,
                                     skip_group_check=True)
                    ps_g.append(ps)
                for g in range(4):  # gate order: f, i, j, o
                    ps = ps_g[g]
                    for mm in range(4):
                        m = g * 4 + mm
                        for k in range(KC):
                            nc.tensor.matmul(
                                ps[:, mm * B:(mm + 1) * B],
                                wh[:, (k * MC + m) * 128:(k * MC + m + 1) * 128],
                                h[:, k * B:(k + 1) * B],
                                start=False, stop=(k == KC - 1),
                                skip_group_check=True,
                            )
                    st = ewpool.tile([128, 4 * B], F32, tag=f"s{g}",
                                     name=f"s{g}")
                    nc.scalar.activation(out=st[:], in_=ps[:],
                                         func=AF.Tanh if g == 2 else AF.Sigmoid)
                    sig[g] = st
                    if g == 0:
                        # on GPSIMD (otherwise idle; SBUF-only operands) so it
                        # runs concurrently with the DVE's u = sig(i)*tanh(j)
                        cf = ewpool.tile([128, 4 * B], F32, tag="cf", name="cf")
                        nc.gpsimd.tensor_tensor(cf[:], c[:], st[:], OP.mult)
                    elif g == 2:
                        u = ewpool.tile([128, 4 * B], F32, tag="u", name="u")
                        nc.vector.tensor_tensor(u[:], sig[1][:], st[:], OP.mult)
                        c_new = stpool.tile([128, KC * B], F32, tag="c",
                                            name="c")
                        nc.vector.tensor_tensor(c_new[:], cf[:], u[:], OP.add)
                        tanh_c = ewpool.tile([128, 4 * B], F32, tag="tc",
                                             name="tc")
                        nc.scalar.activation(out=tanh_c[:], in_=c_new[:],
                                             func=AF.Tanh)
                h_new = stpool.tile([128, KC * B], F16, tag="h", name="h")
                nc.vector.tensor_tensor(h_new[:], tanh_c[:], sig[3][:], OP.mult)
                if t >= W_UP:
                    acc_new = stpool.tile([128, KC * B], F32, tag="acc",
                                          name="acc")
                    nc.gpsimd.tensor_tensor(acc_new[:], acc[:], h_new[:],
                                            OP.add)
                    acc = acc_new
                h, c = h_new, c_new

                if tloc == BS - 1 and bb + 1 < NB:
                    xp_cur = xp_next

            pd = pdpool.tile([B, 1], F32, tag="pd")
            for k in range(KC):
                nc.tensor.matmul(pd[:], acc[:, k * B:(k + 1) * B],
                                 wd[:, k:k + 1],
                                 start=(k == 0), stop=(k == KC - 1))
            res = ewpool.tile([B, 1], F32, tag="res")
            nc.vector.tensor_copy(out=res[:], in_=pd[:])
            nc.sync.dma_start(out=out_d[:], in_=res[:])

    nc.compile()
    return nc


def _get_exec():
    if "exec" in _CACHE:
        return _CACHE["exec"]
    import jax
    import concourse.mybir as mybir
    from concourse import bass2jax
    from jax.sharding import Mesh, PartitionSpec, NamedSharding
    from jax.experimental.shard_map import shard_map

    nc = _build()
    bass2jax.install_neuronx_cc_hook()
    partition_name = (nc.partition_id_tensor.name
                      if nc.partition_id_tensor else None)
    in_names, out_names, out_avals = [], [], []
    for alloc in nc.m.functions[0].allocations:
        if not isinstance(alloc, mybir.MemoryLocationSet):
            continue
        name = alloc.memorylocations[0].name
        if alloc.kind == "ExternalInput":
            if name != partition_name:
                in_names.append(name)
        elif alloc.kind == "ExternalOutput":
            out_names.append(name)
            out_avals.append(jax.core.ShapedArray(
                tuple(alloc.tensor_shape), mybir.dt.np(alloc.dtype)))
    n_params = len(in_names)
    all_in = in_names + out_names + ([partition_name] if partition_name else [])

    def _body(*a):
        operands = list(a)
        if partition_name is not None:
            operands.append(bass2jax.partition_id_tensor())
        return tuple(bass2jax._bass_exec_p.bind(
            *operands, out_avals=tuple(out_avals), in_names=tuple(all_in),
            out_names=tuple(out_names), lowering_input_output_aliases=(),
            sim_require_finite=True, sim_require_nnan=True, nc=nc))

    devices = jax.devices()[:N_CORES]
    mesh = Mesh(np.asarray(devices), ("core",))
    jitted = jax.jit(
        shard_map(_body, mesh=mesh,
                  in_specs=(PartitionSpec("core"),) * (n_params + len(out_avals)),
                  out_specs=(PartitionSpec("core"),) * len(out_names),
                  check_rep=False),
        keep_unused=True)
    shard = NamedSharding(mesh, PartitionSpec("core"))
    state = (jitted, in_names, out_avals, mesh, shard)
    _CACHE["exec"] = state
    return state


def _prep_in_maps(essays, W_lstm, b_lstm, W_dense, b_dense):
    perm = np.concatenate([
        np.arange(1024, 1536),   # f
        np.arange(0, 512),       # i
        np.arange(512, 1024),    # j
        np.arange(1536, 2048),   # o
    ])
    Wx = W_lstm[:D][:, perm]
    Wh = W_lstm[D:][:, perm]
    b_eff = b_lstm[perm].astype(np.float32).copy()
    b_eff[0:512] += 1.0  # TF BasicLSTMCell forget bias ([f] block is first)

    Wx_pad = np.zeros((D_PAD, G4), np.float32)
    Wx_pad[:D] = Wx
    Wx_pad[D] = b_eff  # bias row, matched by constant-1 column in ex
    wx_packed = Wx_pad.reshape(3, 128, MC, 128).transpose(1, 0, 2, 3) \
        .reshape(128, 3 * MC * 128).astype(np.float16)
    import ml_dtypes
    wh_packed = Wh.reshape(KC, 128, MC, 128).transpose(1, 0, 2, 3) \
        .reshape(128, KC * MC * 128).astype(ml_dtypes.float8_e4m3)
    wd_t = W_dense[:, 0].reshape(KC, 128).T.copy().astype(np.float32)

    # global time-padded input: W_UP zero steps (zero state is a fixed
    # point), then essays with the constant-1 bias column
    ex_glob = np.zeros((BF, W_UP + T, D_PAD), np.float16)
    ex_glob[:, W_UP:, :D] = essays.astype(np.float16)
    ex_glob[:, W_UP:, D] = 1.0

    ident = np.eye(128, dtype=np.float16)
    in_maps = []
    for core in range(N_CORES):
        # core's window in padded time coords: [128c, 128c + T_LOC)
        win = ex_glob[:, 128 * core:128 * core + T_LOC]      # [B, T_LOC, 384]
        # -> [k-chunk, d-in-chunk, t, b]
        ex_t = np.ascontiguousarray(
            win.transpose(2, 1, 0).reshape(3, 128, T_LOC * B))
        in_maps.append({
            "ex": ex_t,
            "wh": wh_packed,
            "wx": wx_packed,
            "wd": wd_t,
            "ident": ident,
        })
    return in_maps


def _finish(out, b_dense):
    # out[0]: [N_CORES*B, 1] partial dense dots; sum over cores, mean over
    # time, add bias, sigmoid
    pd = np.asarray(out[0]).reshape(N_CORES, BF).sum(axis=0)
    logits = pd / T + float(b_dense[0])
    return (1.0 / (1.0 + np.exp(-logits))).astype(np.float32)


def kernel(essays, W_lstm, b_lstm, W_dense, b_dense):
    import jax
    essays = np.asarray(essays, np.float32)
    W_lstm = np.asarray(W_lstm, np.float32)
    b_lstm = np.asarray(b_lstm, np.float32)
    W_dense = np.asarray(W_dense, np.float32)
    b_dense = np.asarray(b_dense, np.float32)

    jitted, in_names, out_avals, mesh, shard = _get_exec()
    in_maps = _prep_in_maps(essays, W_lstm, b_lstm, W_dense, b_dense)
    concat_in = [np.concatenate([in_maps[c][nm] for c in range(N_CORES)],
                                axis=0) for nm in in_names]
    concat_zeros = [np.zeros((N_CORES * a.shape[0], *a.shape[1:]), a.dtype)
                    for a in out_avals]
    dev_in = [jax.device_put(a, shard) for a in concat_in]
    dev_zeros = [jax.device_put(a, shard) for a in concat_zeros]
    out = jitted(*dev_in, *dev_zeros)
    jax.block_until_ready(out)
    return _finish(out, b_dense)


# expose the device-resident runner for timing harnesses
def _timed_run(essays, W_lstm, b_lstm, W_dense, b_dense, n_launch=9,
               trials=4):
    """Return (preds, per_launch_seconds_median) using pipelined launches."""
    import time, jax
    jitted, in_names, out_avals, mesh, shard = _get_exec()
    b_dense = np.asarray(b_dense, np.float32)
    in_maps = _prep_in_maps(np.asarray(essays, np.float32),
                            np.asarray(W_lstm, np.float32),
                            np.asarray(b_lstm, np.float32),
                            np.asarray(W_dense, np.float32),
                            b_dense)
    concat_in = [np.concatenate([in_maps[c][nm] for c in range(N_CORES)],
                                axis=0) for nm in in_names]
    concat_zeros = [np.zeros((N_CORES * a.shape[0], *a.shape[1:]), a.dtype)
                    for a in out_avals]
    dev_in = [jax.device_put(a, shard) for a in concat_in]
    dev_zeros = [jax.device_put(a, shard) for a in concat_zeros]

    out = jitted(*dev_in, *dev_zeros)
    jax.block_until_ready(out)
    preds = _finish(out, b_dense)

    def timed(K):
        t0 = time.perf_counter()
        o = None
        for _ in range(K):
            o = jitted(*dev_in, *dev_zeros)
        jax.block_until_ready(o)
        return time.perf_counter() - t0

    # pipelined-launch slope: marginal cost of 16 extra launches.  This is
    # an upper bound on device time (host dispatch overlaps device exec).
    timed(2)  # warm
    margins = []
    for _ in range(trials):
        t3 = timed(3)
        t19 = timed(19)
        margins.append((t19 - t3) / 16)
    return preds, float(np.median(margins))
